# revision 31
# baseline (speedup 1.0000x reference)
# Trainium2 Bass kernel for nn_CoordinateDecoder (self-contained).
#
# Strategy (per core = one batch element, data-parallel over B=8):
#  - Host: sort points by coords[:,0]; compute the 3-level bilinear pyramid
#    in f32 on the host (it is pure input prep); build per-level 2-nnz
#    x-interpolation matrices (y-row weights folded in); positional
#    encoding; fold the FiLM gamma/beta into per-batch MLP weights; then
#    pack EVERYTHING each 1024-token chunk needs (grid rows for all 3
#    levels + interpolation weight blocks + enc/oracle rows) into one
#    contiguous [128, C] f16 stream so the device does ONE input DMA per
#    chunk.
#  - Device: per chunk: stream the pack, sample all 3 levels with grouped
#    matmuls (PSUM-accumulated row pairs), then run the FiLM-folded MLP in
#    transposed activation layout [features, tokens] with bias+gelu fused
#    into the PSUM->SBUF evacuation on the scalar engine.
import sys

if "/opt/trn_rl_repo" not in sys.path:
    sys.path.insert(0, "/opt/trn_rl_repo")

import numpy as np

import concourse.bass as bass
import concourse.mybir as mybir
import concourse.tile as tile
from concourse import bacc
from concourse.bass_utils import run_bass_kernel_spmd

B, H, W, D = 8, 128, 128, 256
N = 8192
NUM_FREQS = 10
MLP_W = 256
DEPTH = 4
NCORES = 8
F16 = mybir.dt.float16
F32 = mybir.dt.float32
GELU = mybir.ActivationFunctionType.Gelu_apprx_tanh
TANH = mybir.ActivationFunctionType.Tanh
TOK_CHUNK = 512


# ----------------------------------------------------------------- host math
def _resize_matrix(in_size: int, out_size: int) -> np.ndarray:
    # port of jax.image.resize(..., 'bilinear', antialias=True) weights
    scale = out_size / in_size
    sample_f = (np.arange(out_size, dtype=np.float64) + 0.5) / scale - 0.5
    x = np.abs(sample_f[None, :] - np.arange(in_size, dtype=np.float64)[:, None]) * scale
    weights = np.maximum(0.0, 1.0 - x)
    total = weights.sum(axis=0, keepdims=True)
    weights = np.where(np.abs(total) > 1000.0 * np.finfo(np.float32).eps, weights / total, 0.0)
    weights = np.where(
        np.logical_and(sample_f[None, :] >= -0.5, sample_f[None, :] <= in_size - 0.5),
        weights, 0.0)
    return weights.astype(np.float32)  # [in, out]


def _positional_encoding(coords: np.ndarray) -> np.ndarray:
    freqs = (2.0 ** np.arange(NUM_FREQS, dtype=np.float32)) * np.float32(np.pi)
    ang = coords[:, None, :] * freqs[None, :, None]
    sc = np.stack([np.sin(ang), np.cos(ang)], axis=2)
    return np.concatenate([coords, sc.reshape(coords.shape[0], -1)], axis=-1).astype(np.float32)


def _sample_prep(coords: np.ndarray, Hl: int, Wl: int):
    n = coords.shape[0]
    y = (coords[:, 0].astype(np.float64) + 1.0) * 0.5 * (Hl - 1)
    x = (coords[:, 1].astype(np.float64) + 1.0) * 0.5 * (Wl - 1)
    y0f = np.clip(np.floor(y), 0.0, Hl - 1)
    x0f = np.clip(np.floor(x), 0.0, Wl - 1)
    y0 = y0f.astype(np.int64)
    x0 = x0f.astype(np.int64)
    wy = (y - y0f).astype(np.float32)
    wx = (x - x0f).astype(np.float32)
    at_edge_y = y0 >= Hl - 1
    y0_eff = np.where(at_edge_y, Hl - 2, y0)
    wa = np.where(at_edge_y, 0.0, 1.0 - wy).astype(np.float32)
    wb = np.where(at_edge_y, 1.0, wy).astype(np.float32)
    at_edge_x = x0 >= Wl - 1
    x0_eff = np.where(at_edge_x, Wl - 2, x0)
    ux0 = np.where(at_edge_x, 0.0, 1.0 - wx).astype(np.float32)
    ux1 = np.where(at_edge_x, 1.0, wx).astype(np.float32)
    u = np.zeros((Wl, n), dtype=np.float32)
    cols = np.arange(n)
    u[x0_eff, cols] = ux0
    u[x0_eff + 1, cols] = ux1
    return y0_eff, u * wa[None, :], u * wb[None, :]


def _chunk_layout(offs):
    """Per-chunk row/block ranges + pack column offsets, derived ONLY from
    offs (so host pack builder and device program agree)."""
    o0, o1, o2 = offs
    n_chunks = N // TOK_CHUNK
    layout = []
    for c in range(n_chunks):
        t0, t1 = c * TOK_CHUNK, (c + 1) * TOK_CHUNK
        ent = {"t0": t0, "t1": t1}
        for li, (o, Hl) in enumerate([(o0, H), (o1, H // 2), (o2, H // 4)]):
            lo = int(np.searchsorted(o[1:], t0, side="right"))
            hi = min(int(np.searchsorted(o[:-1], t1, side="left")), Hl - 2)
            lo = min(lo, Hl - 2)
            ent[f"lo{li}"] = lo
            ent[f"hi{li}"] = hi  # last group index used (groups go to Hl-2)
        # level 0 packs individual rows lo0 .. hi0+1
        n0 = ent["hi0"] - ent["lo0"] + 2
        # levels 1/2 pack row-pair blocks for groups lo..hi
        n1 = ent["hi1"] - ent["lo1"] + 1
        n2 = ent["hi2"] - ent["lo2"] + 1
        ent["n0"], ent["n1"], ent["n2"] = n0, n1, n2
        # section order matches consumption order; sub-DMA split points
        # let each level start sampling as soon as its slice lands
        off = 0
        ent["offua"] = off; off += TOK_CHUNK
        ent["offub"] = off; off += TOK_CHUNK
        ent["off0"] = off; off += n0 * 256
        ent["off1"] = off; off += n1 * 256
        ent["offu1"] = off; off += TOK_CHUNK
        ent["off2"] = off; off += n2 * 256
        ent["offu2"] = off; off += TOK_CHUNK
        ent["offeo"] = off; off += TOK_CHUNK
        ent["cols"] = off
        if c == 0:
            # finer startup split, in ISSUE order: window-0's ua/ub slices
            # and rows go first so the first matmul starts ~4us earlier
            nw0 = min(int(np.searchsorted(o0[:-1], 512, side="left")), H - 2) \
                - ent["lo0"] + 2
            r0e = ent["off0"] + nw0 * 256
            ent["dma_ranges"] = [
                (0, 512), (ent["offub"], ent["offub"] + 512),
                (ent["off0"], r0e),
                (512, ent["offub"]), (ent["offub"] + 512, ent["off0"]),
                (r0e, ent["off1"]), (ent["off1"], ent["off2"]),
                (ent["off2"], off)]
            ent["dma_ranges"] = [(a, b) for a, b in ent["dma_ranges"] if b > a]
        else:
            sp = [0, ent["off0"], ent["off1"], ent["off2"], off]
            ent["dma_ranges"] = list(zip(sp[:-1], sp[1:]))
        layout.append(ent)
    return layout


def _host_prep(inputs: dict):
    coords = np.asarray(inputs["coords"], np.float32)
    context = np.asarray(inputs["context_vector"], np.float32)
    ctx_w = np.asarray(inputs["ctx_w"], np.float32)
    ctx_b = np.asarray(inputs["ctx_b"], np.float32)
    mlp0_w = np.asarray(inputs["mlp0_w"], np.float32)
    mlp0_b = np.asarray(inputs["mlp0_b"], np.float32)
    mlp_hw = np.asarray(inputs["mlp_hw"], np.float32)
    mlp_hb = np.asarray(inputs["mlp_hb"], np.float32)
    out_w = np.asarray(inputs["out_w"], np.float32)
    out_b = np.asarray(inputs["out_b"], np.float32)
    oracle = np.asarray(inputs["oracle_pixels"], np.float32)
    grid = np.asarray(inputs["feature_grid"], np.float32)  # [B, H, W, D]

    perm = np.argsort(coords[:, 0], kind="stable")
    cs = coords[perm]
    enc = _positional_encoding(cs)  # [N, 42]

    offs = []
    us = []
    y0s = []
    for Hl, Wl in [(H, W), (H // 2, W // 2), (H // 4, W // 4)]:
        y0, ua, ub = _sample_prep(cs, Hl, Wl)
        offs.append(np.searchsorted(y0, np.arange(Hl + 1)).astype(np.int64))
        us.append((ua.astype(np.float16), ub.astype(np.float16)))
        y0s.append(y0)

    layout = _chunk_layout(offs)
    totc = sum(e["cols"] for e in layout)

    # host pyramid (f32, exact resize weights)
    ah1 = _resize_matrix(H, H // 2)        # [128, 64]
    aw1 = _resize_matrix(W, W // 2)
    ah2 = _resize_matrix(H, H // 4)        # [128, 32]
    aw2 = _resize_matrix(W, W // 4)

    ctx = context @ ctx_w + ctx_b
    gamma = ctx[:, :MLP_W] + 1.0
    beta = ctx[:, MLP_W:]

    encT = enc.T.astype(np.float16)  # [42, N]
    u1s = np.concatenate([us[1][0], us[1][1]], axis=0)  # [128, N]
    u2s = np.concatenate([us[2][0], us[2][1]], axis=0)  # [64, N]

    per_core = []
    for b in range(B):
        g = grid[b]  # [H, W, D] f32
        # pyramid: y-contract then x-contract, in f32 BLAS
        t1y = (ah1.T @ g.reshape(H, W * D)).reshape(H // 2, W, D)
        l1 = np.einsum("axd,xj->ajd", t1y, aw1, optimize=True)  # [64, 64, 256]
        t2y = (ah2.T @ g.reshape(H, W * D)).reshape(H // 4, W, D)
        l2 = np.einsum("axd,xj->ajd", t2y, aw2, optimize=True)  # [32, 32, 256]
        g16 = g.astype(np.float16)
        l1 = l1.astype(np.float16)
        l2 = l2.astype(np.float16)

        w0 = (mlp0_w * gamma[b][None, :]).astype(np.float16)        # [813, 256]
        # row order [enc | oracle | feats] so enc+oracle form one K=45 part
        w0 = np.concatenate([w0[0:42], w0[810:813], w0[42:810]], axis=0)
        b0 = (mlp0_b * gamma[b] + beta[b]).astype(np.float32)       # [256]
        wh = (mlp_hw * gamma[b][None, None, :]).astype(np.float16)  # [3, 256, 256]
        bh = (mlp_hb * gamma[b][None, :] + beta[b][None, :]).astype(np.float32)
        orcT = np.ascontiguousarray(oracle[b][perm].T).astype(np.float16)  # [3, N]

        pack = np.zeros((128, totc), np.float16)
        base = 0
        for e in layout:
            t0, t1 = e["t0"], e["t1"]
            # level-0 rows: g16[r] is [W=128, 256] with x on partitions
            r0 = e["lo0"]
            for i in range(e["n0"]):
                pack[:, base + e["off0"] + i * 256: base + e["off0"] + (i + 1) * 256] = g16[r0 + i]
            # level-1 row-pair blocks (64 partitions row r, 64 row r+1)
            for i in range(e["n1"]):
                r = e["lo1"] + i
                col = base + e["off1"] + i * 256
                pack[0:64, col:col + 256] = l1[r]
                pack[64:128, col:col + 256] = l1[r + 1]
            # level-2 row-pair blocks on partitions 0:64
            for i in range(e["n2"]):
                r = e["lo2"] + i
                col = base + e["off2"] + i * 256
                pack[0:32, col:col + 256] = l2[r]
                pack[32:64, col:col + 256] = l2[r + 1]
            pack[:, base + e["offua"]: base + e["offua"] + TOK_CHUNK] = us[0][0][:, t0:t1]
            pack[:, base + e["offub"]: base + e["offub"] + TOK_CHUNK] = us[0][1][:, t0:t1]
            pack[:, base + e["offu1"]: base + e["offu1"] + TOK_CHUNK] = u1s[:, t0:t1]
            pack[0:64, base + e["offu2"]: base + e["offu2"] + TOK_CHUNK] = u2s[:, t0:t1]
            pack[0:42, base + e["offeo"]: base + e["offeo"] + TOK_CHUNK] = encT[:, t0:t1]
            pack[42:45, base + e["offeo"]: base + e["offeo"] + TOK_CHUNK] = orcT[:, t0:t1]
            base += e["cols"]

        # weights pack: w0 7 tiles | wh 6 tiles | wout 2 tiles  (f16)
        wpack = np.zeros((128, 13 * 256 + 8), np.float16)
        w0_bounds = [0, 45, 173, 301, 429, 557, 685, 813]
        for i in range(7):
            lo, hi = w0_bounds[i], w0_bounds[i + 1]
            wpack[0:hi - lo, i * 256:(i + 1) * 256] = w0[lo:hi]
        for l in range(3):
            for k in range(2):
                wpack[:, (7 + l * 2 + k) * 256:(8 + l * 2 + k) * 256] = wh[l, k * 128:(k + 1) * 128]
        wout16 = (out_w * 1.0).astype(np.float16)  # [256, 3]
        wpack[:, 13 * 256 + 0:13 * 256 + 3] = wout16[0:128]
        wpack[:, 13 * 256 + 3:13 * 256 + 6] = wout16[128:256]

        bpack = np.zeros((128, 9), np.float32)
        bpack[:, 0:2] = np.ascontiguousarray(b0.reshape(2, 128).T)
        bpack[:, 2:8] = bh.reshape(3, 2, 128).transpose(2, 0, 1).reshape(128, 6)
        bpack[0:3, 8] = out_b

        per_core.append({"pack": pack, "wpack": wpack, "bpack": bpack})
    return per_core, perm, offs, totc


# ------------------------------------------------------------- device kernel
def _build_program(offs, totc):
    nc = bacc.Bacc("TRN2", target_bir_lowering=False, debug=False, num_devices=NCORES)

    pack = nc.dram_tensor("pack", [128, totc], F16, kind="ExternalInput")
    wpack = nc.dram_tensor("wpack", [128, 13 * 256 + 8], F16, kind="ExternalInput")
    bpack = nc.dram_tensor("bpack", [128, 9], F32, kind="ExternalInput")
    out_t = nc.dram_tensor("out_t", [3, N], F32, kind="ExternalOutput")

    o0, o1, o2 = offs
    layout = _chunk_layout(offs)
    n_chunks = N // TOK_CHUNK

    with tile.TileContext(nc) as tc:
        with tc.tile_pool(name="persist", bufs=1) as persist:
            # weight DMAs are issued inside the chunk loop (after chunk 0's
            # stream DMAs) so sampling isn't stuck behind them at startup
            w_sb = persist.tile([128, 13 * 256 + 8], F16, tag="w")
            b_sb = persist.tile([128, 9], F32, tag="b")

            w0_bounds = [0, 45, 173, 301, 429, 557, 685, 813]
            w0_parts = [w_sb[0:w0_bounds[i + 1] - w0_bounds[i], i * 256:(i + 1) * 256]
                        for i in range(7)]
            wh_parts = [[w_sb[:, (7 + l * 2 + k) * 256:(8 + l * 2 + k) * 256]
                         for k in range(2)] for l in range(3)]
            wout_parts = [w_sb[:, 13 * 256 + 3 * k:13 * 256 + 3 * (k + 1)]
                          for k in range(2)]
            b0_sb = b_sb[:, 0:2]
            bh_sb = b_sb[:, 2:8]
            bout_sb = b_sb[0:3, 8:9]

            psum_cm = tc.tile_pool(name="psumS", bufs=4, space="PSUM")
            psum_mlp_cm = tc.tile_pool(name="psumM", bufs=4, space="PSUM")
            with psum_cm as psum, psum_mlp_cm as psum_mlp:

                def level_sample(offv, n_rows_l, ua, ub, row_of, dest, t0, t1,
                                 evac=None):
                    """Grouped-matmul sampling of one level for tokens
                    [t0, t1); dest maps hf -> [128, TOK_CHUNK] f16 tile.
                    Groups are packed into 512-token PSUM windows."""
                    for w0c in range(t0, t1, 512):
                        w1c = w0c + 512
                        glo = int(np.searchsorted(offv[1:], w0c, side="right"))
                        for hf in range(2):
                            ps = psum.tile([128, 512], F32, tag="ps")
                            r = glo
                            first = True
                            while r < n_rows_l - 1 and int(offv[r]) < w1c:
                                s0 = max(int(offv[r]), w0c)
                                s1 = min(int(offv[r + 1]), w1c)
                                if s1 > s0:
                                    la, lb = s0 - w0c, s1 - w0c
                                    if ub is not None:
                                        nc.tensor.matmul(
                                            out=ps[:, la:lb],
                                            lhsT=row_of(r)[:, hf * 128:(hf + 1) * 128],
                                            rhs=ua[:, s0 - t0:s1 - t0],
                                            start=True, stop=False)
                                        nc.tensor.matmul(
                                            out=ps[:, la:lb],
                                            lhsT=row_of(r + 1)[:, hf * 128:(hf + 1) * 128],
                                            rhs=ub[:, s0 - t0:s1 - t0],
                                            start=False, stop=True)
                                    else:
                                        nc.tensor.matmul(
                                            out=ps[:, la:lb],
                                            lhsT=row_of(r)[:, hf * 128:(hf + 1) * 128],
                                            rhs=ua[:, s0 - t0:s1 - t0],
                                            start=True, stop=True)
                                    first = False
                                r += 1
                            if first:
                                continue
                            if evac == "scalar":
                                nc.scalar.copy(
                                    out=dest[hf][:, w0c - t0:w1c - t0], in_=ps)
                            else:
                                nc.vector.tensor_copy(
                                    out=dest[hf][:, w0c - t0:w1c - t0], in_=ps)

                def sampling_thunks_for(offv, n_rows_l, ua, ub, row_of, dest,
                                        t0, t1, evac=None):
                    """Like level_sample but returns one thunk per
                    (window, hf) for software-pipelined emission."""
                    thunks = []
                    for w0c in range(t0, t1, 512):
                        for hf in range(2):
                            def unit(w0c=w0c, hf=hf):
                                w1c = w0c + 512
                                glo = int(np.searchsorted(offv[1:], w0c,
                                                          side="right"))
                                ps = psum.tile([128, 512], F32, tag="ps")
                                r = glo
                                first = True
                                while r < n_rows_l - 1 and int(offv[r]) < w1c:
                                    s0 = max(int(offv[r]), w0c)
                                    s1 = min(int(offv[r + 1]), w1c)
                                    if s1 > s0:
                                        la, lb = s0 - w0c, s1 - w0c
                                        if ub is not None:
                                            nc.tensor.matmul(
                                                out=ps[:, la:lb],
                                                lhsT=row_of(r)[:, hf * 128:(hf + 1) * 128],
                                                rhs=ua[:, s0 - t0:s1 - t0],
                                                start=True, stop=False)
                                            nc.tensor.matmul(
                                                out=ps[:, la:lb],
                                                lhsT=row_of(r + 1)[:, hf * 128:(hf + 1) * 128],
                                                rhs=ub[:, s0 - t0:s1 - t0],
                                                start=False, stop=True)
                                        else:
                                            nc.tensor.matmul(
                                                out=ps[:, la:lb],
                                                lhsT=row_of(r)[:, hf * 128:(hf + 1) * 128],
                                                rhs=ua[:, s0 - t0:s1 - t0],
                                                start=True, stop=True)
                                        first = False
                                    r += 1
                                if first:
                                    return
                                if evac == "scalar":
                                    nc.scalar.copy(
                                        out=dest[hf][:, w0c - t0:w1c - t0],
                                        in_=ps)
                                else:
                                    nc.vector.tensor_copy(
                                        out=dest[hf][:, w0c - t0:w1c - t0],
                                        in_=ps)
                            thunks.append(unit)
                    return thunks

                with tc.tile_pool(name="pk", bufs=3) as pk_pool, \
                     tc.tile_pool(name="s0c", bufs=2) as s0c_pool, \
                     tc.tile_pool(name="schunk", bufs=2) as schunk, \
                     tc.tile_pool(name="hchunk", bufs=2) as hchunk, \
                     tc.tile_pool(name="ochunk", bufs=2) as ochunk:
                    maxc = max(e["cols"] for e in layout)
                    n_sub = TOK_CHUNK // 512

                    def issue_dma(c, base):
                        e = layout[c]
                        pk = pk_pool.tile([128, maxc], F16, tag="pk", name="pk")
                        for sa, sb in e["dma_ranges"]:
                            nc.sync.dma_start(
                                out=pk[:, sa:sb],
                                in_=pack[:, base + sa:base + sb])
                        if c == 0:
                            nc.sync.dma_start(out=w_sb, in_=wpack[:, :])
                            nc.sync.dma_start(out=b_sb, in_=bpack[:, :])
                        return pk

                    def build_sampling(c, pk):
                        """Allocate dest tiles and build sampling thunks
                        for chunk c (data already streaming into pk)."""
                        e = layout[c]
                        t0, t1 = e["t0"], e["t1"]
                        ua0 = pk[:, e["offua"]:e["offua"] + TOK_CHUNK]
                        ub0 = pk[:, e["offub"]:e["offub"] + TOK_CHUNK]
                        u1v = pk[:, e["offu1"]:e["offu1"] + TOK_CHUNK]
                        u2v = pk[0:64, e["offu2"]:e["offu2"] + TOK_CHUNK]
                        encorc = pk[0:45, e["offeo"]:e["offeo"] + TOK_CHUNK]

                        def row0(r, e=e, pk=pk):
                            i = r - e["lo0"]
                            return pk[:, e["off0"] + i * 256:e["off0"] + (i + 1) * 256]

                        def row1(r, e=e, pk=pk):
                            i = r - e["lo1"]
                            return pk[:, e["off1"] + i * 256:e["off1"] + (i + 1) * 256]

                        def row2(r, e=e, pk=pk):
                            i = r - e["lo2"]
                            return pk[0:64, e["off2"] + i * 256:e["off2"] + (i + 1) * 256]

                        s0t = [s0c_pool.tile([128, TOK_CHUNK], F16,
                                             tag=f"s0t{hf}", name=f"s0t{hf}")
                               for hf in range(2)]
                        s_sb = {}
                        for lvl in (1, 2):
                            for hf in range(2):
                                s_sb[(lvl, hf)] = schunk.tile(
                                    [128, TOK_CHUNK], F16, tag=f"s{lvl}{hf}",
                                    name=f"s{lvl}{hf}")
                        thunks = (
                            sampling_thunks_for(o0, H, ua0, ub0, row0,
                                                {0: s0t[0], 1: s0t[1]}, t0, t1)
                            + sampling_thunks_for(o1, H // 2, u1v, None, row1,
                                                  {0: s_sb[(1, 0)], 1: s_sb[(1, 1)]},
                                                  t0, t1)
                            + sampling_thunks_for(o2, H // 4, u2v, None, row2,
                                                  {0: s_sb[(2, 0)], 1: s_sb[(2, 1)]},
                                                  t0, t1, evac="scalar"))
                        l1_rhs = [encorc, s0t[0], s0t[1], s_sb[(1, 0)],
                                  s_sb[(1, 1)], s_sb[(2, 0)], s_sb[(2, 1)]]
                        return thunks, (t0, t1, l1_rhs)

                    def build_mlp(ctx):
                        """Return MLP thunk list for a sampled chunk."""
                        t0, t1, l1_rhs = ctx
                        h_cur = [hchunk.tile([128, TOK_CHUNK], F16,
                                             tag=f"h{hf}", name=f"h{hf}")
                                 for hf in range(2)]
                        thunks = []
                        for hf in range(2):
                            for s in range(n_sub):
                                def unit(hf=hf, s=s, h_cur=h_cur, l1_rhs=l1_rhs):
                                    ps = psum_mlp.tile([128, 512], F32, tag="ps")
                                    for k in range(7):
                                        nc.tensor.matmul(
                                            out=ps,
                                            lhsT=w0_parts[k][:, hf * 128:(hf + 1) * 128],
                                            rhs=l1_rhs[k][:, s * 512:(s + 1) * 512],
                                            start=(k == 0), stop=(k == 6))
                                    nc.scalar.activation(
                                        out=h_cur[hf][:, s * 512:(s + 1) * 512],
                                        in_=ps, func=GELU,
                                        bias=b0_sb[:, hf:hf + 1])
                                thunks.append(unit)
                        for layer in range(DEPTH - 1):
                            h_nxt = [hchunk.tile([128, TOK_CHUNK], F16,
                                                 tag=f"hn{layer % 2}{hf}",
                                                 name=f"hn{layer % 2}{hf}")
                                     for hf in range(2)]
                            for hf in range(2):
                                for s in range(n_sub):
                                    def unit(layer=layer, hf=hf, s=s,
                                             h_cur=h_cur, h_nxt=h_nxt):
                                        ps = psum_mlp.tile([128, 512], F32,
                                                           tag="ps")
                                        for k in range(2):
                                            nc.tensor.matmul(
                                                out=ps,
                                                lhsT=wh_parts[layer][k][:, hf * 128:(hf + 1) * 128],
                                                rhs=h_cur[k][:, s * 512:(s + 1) * 512],
                                                start=(k == 0), stop=(k == 1))
                                        nc.scalar.activation(
                                            out=h_nxt[hf][:, s * 512:(s + 1) * 512],
                                            in_=ps, func=GELU,
                                            bias=bh_sb[:, layer * 2 + hf:layer * 2 + hf + 1])
                                    thunks.append(unit)
                            h_cur = h_nxt
                        oc = ochunk.tile([3, TOK_CHUNK], F32, tag="oc")
                        for s in range(n_sub):
                            def unit(s=s, h_cur=h_cur, oc=oc, t0=t0, t1=t1):
                                ps = psum_mlp.tile([3, 512], F32, tag="ps")
                                for k in range(2):
                                    nc.tensor.matmul(
                                        out=ps, lhsT=wout_parts[k][0:128, :],
                                        rhs=h_cur[k][:, s * 512:(s + 1) * 512],
                                        start=(k == 0), stop=(k == 1))
                                nc.vector.tensor_copy(
                                    out=oc[:, s * 512:(s + 1) * 512], in_=ps)
                                if s == n_sub - 1:
                                    nc.sync.dma_start(out=out_t[:, t0:t1],
                                                      in_=oc)
                            thunks.append(unit)
                        return thunks

                    def weave(samp, mlp):
                        """Emit mlp units (prev chunk) interleaved with
                        sampling units (current chunk) so the in-order PE
                        queue has filler during activation latency."""
                        for u in mlp:
                            u()
                        for u in samp:
                            u()

                    bases = [0]
                    for e in layout:
                        bases.append(bases[-1] + e["cols"])
                    prev_mlp = []
                    pks = {0: issue_dma(0, bases[0])}
                    for c in range(n_chunks):
                        if c + 1 < n_chunks:
                            pks[c + 1] = issue_dma(c + 1, bases[c + 1])
                        samp, ctx = build_sampling(c, pks.pop(c))
                        weave(samp, prev_mlp)
                        prev_mlp = build_mlp(ctx)
                    weave([], prev_mlp)

    nc.compile()
    return nc


# ------------------------------------------------------------------ wrapper
_cache = {}


def kernel(**inputs) -> np.ndarray:
    per_core, perm, offs, totc = _host_prep(inputs)
    key = (totc,) + tuple(tuple(int(v) for v in o) for o in offs)
    if key not in _cache:
        _cache.clear()
        _cache[key] = _build_program(offs, totc)
    nc = _cache[key]
    res = run_bass_kernel_spmd(nc, per_core, core_ids=list(range(NCORES)))
    out = np.zeros((B, N, 3), np.float32)
    out_b = np.asarray(inputs["out_b"], np.float32)
    for b in range(B):
        # device returns pre-activation z = h3 @ wout; finish on host
        out[b, perm] = np.tanh(res.results[b]["out_t"].T + out_b[None, :])
    return out


if __name__ == "__main__":
    rng = np.random.default_rng(0)
    inputs = {
        "feature_grid": rng.standard_normal((B, H, W, D), dtype=np.float32),
        "context_vector": rng.standard_normal((B, D), dtype=np.float32),
        "coords": rng.uniform(-1, 1, (N, 2)).astype(np.float32),
        "oracle_pixels": rng.uniform(0, 1, (B, N, 3)).astype(np.float32),
        "mlp0_w": (rng.standard_normal((813, 256)) / np.sqrt(813)).astype(np.float32),
        "mlp0_b": np.zeros(256, np.float32),
        "mlp_hw": (rng.standard_normal((3, 256, 256)) / 16).astype(np.float32),
        "mlp_hb": np.zeros((3, 256), np.float32),
        "ctx_w": (rng.standard_normal((256, 512)) / 16).astype(np.float32),
        "ctx_b": np.zeros(512, np.float32),
        "out_w": (rng.standard_normal((256, 3)) / 16 * 0.01).astype(np.float32),
        "out_b": np.zeros(3, np.float32),
    }
    out = kernel(**inputs)
    print("kernel out:", out.shape, out.dtype, np.abs(out).max())


# revision 33
# speedup vs baseline: 1.0051x; 1.0051x over previous
# Trainium2 Bass kernel for nn_CoordinateDecoder (self-contained).
#
# Strategy (per core = one batch element, data-parallel over B=8):
#  - Host: sort points by coords[:,0]; compute the 3-level bilinear pyramid
#    in f32 on the host (it is pure input prep); build per-level 2-nnz
#    x-interpolation matrices (y-row weights folded in); positional
#    encoding; fold the FiLM gamma/beta into per-batch MLP weights; then
#    pack EVERYTHING each 1024-token chunk needs (grid rows for all 3
#    levels + interpolation weight blocks + enc/oracle rows) into one
#    contiguous [128, C] f16 stream so the device does ONE input DMA per
#    chunk.
#  - Device: per chunk: stream the pack, sample all 3 levels with grouped
#    matmuls (PSUM-accumulated row pairs), then run the FiLM-folded MLP in
#    transposed activation layout [features, tokens] with bias+gelu fused
#    into the PSUM->SBUF evacuation on the scalar engine.
import sys

if "/opt/trn_rl_repo" not in sys.path:
    sys.path.insert(0, "/opt/trn_rl_repo")

import numpy as np

import concourse.bass as bass
import concourse.mybir as mybir
import concourse.tile as tile
from concourse import bacc
from concourse.bass_utils import run_bass_kernel_spmd

B, H, W, D = 8, 128, 128, 256
N = 8192
NUM_FREQS = 10
MLP_W = 256
DEPTH = 4
NCORES = 8
F16 = mybir.dt.float16
F32 = mybir.dt.float32
GELU = mybir.ActivationFunctionType.Gelu_apprx_tanh
TANH = mybir.ActivationFunctionType.Tanh
TOK_CHUNK = 512


# ----------------------------------------------------------------- host math
def _resize_matrix(in_size: int, out_size: int) -> np.ndarray:
    # port of jax.image.resize(..., 'bilinear', antialias=True) weights
    scale = out_size / in_size
    sample_f = (np.arange(out_size, dtype=np.float64) + 0.5) / scale - 0.5
    x = np.abs(sample_f[None, :] - np.arange(in_size, dtype=np.float64)[:, None]) * scale
    weights = np.maximum(0.0, 1.0 - x)
    total = weights.sum(axis=0, keepdims=True)
    weights = np.where(np.abs(total) > 1000.0 * np.finfo(np.float32).eps, weights / total, 0.0)
    weights = np.where(
        np.logical_and(sample_f[None, :] >= -0.5, sample_f[None, :] <= in_size - 0.5),
        weights, 0.0)
    return weights.astype(np.float32)  # [in, out]


def _positional_encoding(coords: np.ndarray) -> np.ndarray:
    freqs = (2.0 ** np.arange(NUM_FREQS, dtype=np.float32)) * np.float32(np.pi)
    ang = coords[:, None, :] * freqs[None, :, None]
    sc = np.stack([np.sin(ang), np.cos(ang)], axis=2)
    return np.concatenate([coords, sc.reshape(coords.shape[0], -1)], axis=-1).astype(np.float32)


def _sample_prep(coords: np.ndarray, Hl: int, Wl: int):
    n = coords.shape[0]
    y = (coords[:, 0].astype(np.float64) + 1.0) * 0.5 * (Hl - 1)
    x = (coords[:, 1].astype(np.float64) + 1.0) * 0.5 * (Wl - 1)
    y0f = np.clip(np.floor(y), 0.0, Hl - 1)
    x0f = np.clip(np.floor(x), 0.0, Wl - 1)
    y0 = y0f.astype(np.int64)
    x0 = x0f.astype(np.int64)
    wy = (y - y0f).astype(np.float32)
    wx = (x - x0f).astype(np.float32)
    at_edge_y = y0 >= Hl - 1
    y0_eff = np.where(at_edge_y, Hl - 2, y0)
    wa = np.where(at_edge_y, 0.0, 1.0 - wy).astype(np.float32)
    wb = np.where(at_edge_y, 1.0, wy).astype(np.float32)
    at_edge_x = x0 >= Wl - 1
    x0_eff = np.where(at_edge_x, Wl - 2, x0)
    ux0 = np.where(at_edge_x, 0.0, 1.0 - wx).astype(np.float32)
    ux1 = np.where(at_edge_x, 1.0, wx).astype(np.float32)
    u = np.zeros((Wl, n), dtype=np.float32)
    cols = np.arange(n)
    u[x0_eff, cols] = ux0
    u[x0_eff + 1, cols] = ux1
    return y0_eff, u * wa[None, :], u * wb[None, :]


def _chunk_layout(offs):
    """Per-chunk row/block ranges + pack column offsets, derived ONLY from
    offs (so host pack builder and device program agree)."""
    o0, o1, o2 = offs
    n_chunks = N // TOK_CHUNK
    layout = []
    for c in range(n_chunks):
        t0, t1 = c * TOK_CHUNK, (c + 1) * TOK_CHUNK
        ent = {"t0": t0, "t1": t1}
        for li, (o, Hl) in enumerate([(o0, H), (o1, H // 2), (o2, H // 4)]):
            lo = int(np.searchsorted(o[1:], t0, side="right"))
            hi = min(int(np.searchsorted(o[:-1], t1, side="left")), Hl - 2)
            lo = min(lo, Hl - 2)
            ent[f"lo{li}"] = lo
            ent[f"hi{li}"] = hi  # last group index used (groups go to Hl-2)
        # level 0 packs individual rows lo0 .. hi0+1
        n0 = ent["hi0"] - ent["lo0"] + 2
        # levels 1/2 pack row-pair blocks for groups lo..hi
        n1 = ent["hi1"] - ent["lo1"] + 1
        n2 = ent["hi2"] - ent["lo2"] + 1
        ent["n0"], ent["n1"], ent["n2"] = n0, n1, n2
        # section order matches consumption order; sub-DMA split points
        # let each level start sampling as soon as its slice lands
        off = 0
        ent["offua"] = off; off += TOK_CHUNK
        ent["offub"] = off; off += TOK_CHUNK
        ent["off0"] = off; off += n0 * 256
        ent["off1"] = off; off += n1 * 256
        ent["offu1"] = off; off += TOK_CHUNK
        ent["off2"] = off; off += n2 * 256
        ent["offu2"] = off; off += TOK_CHUNK
        ent["offeo"] = ent["offu2"]
        ent["cols"] = off
        if c == 0:
            # finer startup split, in ISSUE order: window-0's ua/ub slices
            # and rows go first so the first matmul starts ~4us earlier
            nw0 = min(int(np.searchsorted(o0[:-1], 512, side="left")), H - 2) \
                - ent["lo0"] + 2
            r0e = ent["off0"] + nw0 * 256
            ent["dma_ranges"] = [
                (0, 512), (ent["offub"], ent["offub"] + 512),
                (ent["off0"], r0e),
                (512, ent["offub"]), (ent["offub"] + 512, ent["off0"]),
                (r0e, ent["off1"]), (ent["off1"], ent["off2"]),
                (ent["off2"], off)]
            ent["dma_ranges"] = [(a, b) for a, b in ent["dma_ranges"] if b > a]
        else:
            sp = [0, ent["off0"], ent["off1"], ent["off2"], off]
            ent["dma_ranges"] = list(zip(sp[:-1], sp[1:]))
        layout.append(ent)
    return layout


def _host_prep(inputs: dict):
    coords = np.asarray(inputs["coords"], np.float32)
    context = np.asarray(inputs["context_vector"], np.float32)
    ctx_w = np.asarray(inputs["ctx_w"], np.float32)
    ctx_b = np.asarray(inputs["ctx_b"], np.float32)
    mlp0_w = np.asarray(inputs["mlp0_w"], np.float32)
    mlp0_b = np.asarray(inputs["mlp0_b"], np.float32)
    mlp_hw = np.asarray(inputs["mlp_hw"], np.float32)
    mlp_hb = np.asarray(inputs["mlp_hb"], np.float32)
    out_w = np.asarray(inputs["out_w"], np.float32)
    out_b = np.asarray(inputs["out_b"], np.float32)
    oracle = np.asarray(inputs["oracle_pixels"], np.float32)
    grid = np.asarray(inputs["feature_grid"], np.float32)  # [B, H, W, D]

    perm = np.argsort(coords[:, 0], kind="stable")
    cs = coords[perm]
    enc = _positional_encoding(cs)  # [N, 42]

    offs = []
    us = []
    y0s = []
    for Hl, Wl in [(H, W), (H // 2, W // 2), (H // 4, W // 4)]:
        y0, ua, ub = _sample_prep(cs, Hl, Wl)
        offs.append(np.searchsorted(y0, np.arange(Hl + 1)).astype(np.int64))
        us.append((ua.astype(np.float16), ub.astype(np.float16)))
        y0s.append(y0)

    layout = _chunk_layout(offs)
    totc = sum(e["cols"] for e in layout)

    # host pyramid (f32, exact resize weights)
    ah1 = _resize_matrix(H, H // 2)        # [128, 64]
    aw1 = _resize_matrix(W, W // 2)
    ah2 = _resize_matrix(H, H // 4)        # [128, 32]
    aw2 = _resize_matrix(W, W // 4)

    ctx = context @ ctx_w + ctx_b
    gamma = ctx[:, :MLP_W] + 1.0
    beta = ctx[:, MLP_W:]

    encT = enc.T.astype(np.float16)  # [42, N]
    u1s = np.concatenate([us[1][0], us[1][1]], axis=0)  # [128, N]
    u2s = np.concatenate([us[2][0], us[2][1]], axis=0)  # [64, N]

    per_core = []
    for b in range(B):
        g = grid[b]  # [H, W, D] f32
        # pyramid: y-contract then x-contract, in f32 BLAS
        t1y = (ah1.T @ g.reshape(H, W * D)).reshape(H // 2, W, D)
        l1 = np.einsum("axd,xj->ajd", t1y, aw1, optimize=True)  # [64, 64, 256]
        t2y = (ah2.T @ g.reshape(H, W * D)).reshape(H // 4, W, D)
        l2 = np.einsum("axd,xj->ajd", t2y, aw2, optimize=True)  # [32, 32, 256]
        g16 = g.astype(np.float16)
        l1 = l1.astype(np.float16)
        l2 = l2.astype(np.float16)

        w0 = (mlp0_w * gamma[b][None, :]).astype(np.float16)        # [813, 256]
        # row order [enc | oracle | feats] so enc+oracle form one K=45 part
        w0 = np.concatenate([w0[0:42], w0[810:813], w0[42:810]], axis=0)
        b0 = (mlp0_b * gamma[b] + beta[b]).astype(np.float32)       # [256]
        wh = (mlp_hw * gamma[b][None, None, :]).astype(np.float16)  # [3, 256, 256]
        bh = (mlp_hb * gamma[b][None, :] + beta[b][None, :]).astype(np.float32)
        orcT = np.ascontiguousarray(oracle[b][perm].T).astype(np.float16)  # [3, N]

        pack = np.zeros((128, totc), np.float16)
        base = 0
        for e in layout:
            t0, t1 = e["t0"], e["t1"]
            # level-0 rows: g16[r] is [W=128, 256] with x on partitions
            r0 = e["lo0"]
            for i in range(e["n0"]):
                pack[:, base + e["off0"] + i * 256: base + e["off0"] + (i + 1) * 256] = g16[r0 + i]
            # level-1 row-pair blocks (64 partitions row r, 64 row r+1)
            for i in range(e["n1"]):
                r = e["lo1"] + i
                col = base + e["off1"] + i * 256
                pack[0:64, col:col + 256] = l1[r]
                pack[64:128, col:col + 256] = l1[r + 1]
            # level-2 row-pair blocks on partitions 0:64
            for i in range(e["n2"]):
                r = e["lo2"] + i
                col = base + e["off2"] + i * 256
                pack[0:32, col:col + 256] = l2[r]
                pack[32:64, col:col + 256] = l2[r + 1]
            pack[:, base + e["offua"]: base + e["offua"] + TOK_CHUNK] = us[0][0][:, t0:t1]
            pack[:, base + e["offub"]: base + e["offub"] + TOK_CHUNK] = us[0][1][:, t0:t1]
            pack[:, base + e["offu1"]: base + e["offu1"] + TOK_CHUNK] = u1s[:, t0:t1]
            pack[0:64, base + e["offu2"]: base + e["offu2"] + TOK_CHUNK] = u2s[:, t0:t1]
            pack[64:106, base + e["offeo"]: base + e["offeo"] + TOK_CHUNK] = encT[:, t0:t1]
            pack[106:109, base + e["offeo"]: base + e["offeo"] + TOK_CHUNK] = orcT[:, t0:t1]
            base += e["cols"]

        # weights pack: w0 7 tiles | wh 6 tiles | wout 2 tiles  (f16)
        wpack = np.zeros((128, 13 * 256 + 8), np.float16)
        w0_bounds = [0, 45, 173, 301, 429, 557, 685, 813]
        for i in range(7):
            lo, hi = w0_bounds[i], w0_bounds[i + 1]
            pbase = 64 if i == 0 else 0
            wpack[pbase:pbase + hi - lo, i * 256:(i + 1) * 256] = w0[lo:hi]
        for l in range(3):
            for k in range(2):
                wpack[:, (7 + l * 2 + k) * 256:(8 + l * 2 + k) * 256] = wh[l, k * 128:(k + 1) * 128]
        wout16 = (out_w * 1.0).astype(np.float16)  # [256, 3]
        wpack[:, 13 * 256 + 0:13 * 256 + 3] = wout16[0:128]
        wpack[:, 13 * 256 + 3:13 * 256 + 6] = wout16[128:256]

        bpack = np.zeros((128, 9), np.float32)
        bpack[:, 0:2] = np.ascontiguousarray(b0.reshape(2, 128).T)
        bpack[:, 2:8] = bh.reshape(3, 2, 128).transpose(2, 0, 1).reshape(128, 6)
        bpack[0:3, 8] = out_b

        per_core.append({"pack": pack, "wpack": wpack, "bpack": bpack})
    return per_core, perm, offs, totc


# ------------------------------------------------------------- device kernel
def _build_program(offs, totc):
    nc = bacc.Bacc("TRN2", target_bir_lowering=False, debug=False, num_devices=NCORES)

    pack = nc.dram_tensor("pack", [128, totc], F16, kind="ExternalInput")
    wpack = nc.dram_tensor("wpack", [128, 13 * 256 + 8], F16, kind="ExternalInput")
    bpack = nc.dram_tensor("bpack", [128, 9], F32, kind="ExternalInput")
    out_t = nc.dram_tensor("out_t", [3, N], F32, kind="ExternalOutput")

    o0, o1, o2 = offs
    layout = _chunk_layout(offs)
    n_chunks = N // TOK_CHUNK

    with tile.TileContext(nc) as tc:
        with tc.tile_pool(name="persist", bufs=1) as persist:
            # weight DMAs are issued inside the chunk loop (after chunk 0's
            # stream DMAs) so sampling isn't stuck behind them at startup
            w_sb = persist.tile([128, 13 * 256 + 8], F16, tag="w")
            b_sb = persist.tile([128, 9], F32, tag="b")

            w0_bounds = [0, 45, 173, 301, 429, 557, 685, 813]
            w0_parts = [w_sb[(64 if i == 0 else 0):
                             (64 if i == 0 else 0) + w0_bounds[i + 1] - w0_bounds[i],
                             i * 256:(i + 1) * 256]
                        for i in range(7)]
            wh_parts = [[w_sb[:, (7 + l * 2 + k) * 256:(8 + l * 2 + k) * 256]
                         for k in range(2)] for l in range(3)]
            wout_parts = [w_sb[:, 13 * 256 + 3 * k:13 * 256 + 3 * (k + 1)]
                          for k in range(2)]
            b0_sb = b_sb[:, 0:2]
            bh_sb = b_sb[:, 2:8]
            bout_sb = b_sb[0:3, 8:9]

            psum_cm = tc.tile_pool(name="psumS", bufs=4, space="PSUM")
            psum_mlp_cm = tc.tile_pool(name="psumM", bufs=4, space="PSUM")
            with psum_cm as psum, psum_mlp_cm as psum_mlp:

                def level_sample(offv, n_rows_l, ua, ub, row_of, dest, t0, t1,
                                 evac=None):
                    """Grouped-matmul sampling of one level for tokens
                    [t0, t1); dest maps hf -> [128, TOK_CHUNK] f16 tile.
                    Groups are packed into 512-token PSUM windows."""
                    for w0c in range(t0, t1, 512):
                        w1c = w0c + 512
                        glo = int(np.searchsorted(offv[1:], w0c, side="right"))
                        for hf in range(2):
                            ps = psum.tile([128, 512], F32, tag="ps")
                            r = glo
                            first = True
                            while r < n_rows_l - 1 and int(offv[r]) < w1c:
                                s0 = max(int(offv[r]), w0c)
                                s1 = min(int(offv[r + 1]), w1c)
                                if s1 > s0:
                                    la, lb = s0 - w0c, s1 - w0c
                                    if ub is not None:
                                        nc.tensor.matmul(
                                            out=ps[:, la:lb],
                                            lhsT=row_of(r)[:, hf * 128:(hf + 1) * 128],
                                            rhs=ua[:, s0 - t0:s1 - t0],
                                            start=True, stop=False)
                                        nc.tensor.matmul(
                                            out=ps[:, la:lb],
                                            lhsT=row_of(r + 1)[:, hf * 128:(hf + 1) * 128],
                                            rhs=ub[:, s0 - t0:s1 - t0],
                                            start=False, stop=True)
                                    else:
                                        nc.tensor.matmul(
                                            out=ps[:, la:lb],
                                            lhsT=row_of(r)[:, hf * 128:(hf + 1) * 128],
                                            rhs=ua[:, s0 - t0:s1 - t0],
                                            start=True, stop=True)
                                    first = False
                                r += 1
                            if first:
                                continue
                            if evac == "scalar":
                                nc.scalar.copy(
                                    out=dest[hf][:, w0c - t0:w1c - t0], in_=ps)
                            else:
                                nc.vector.tensor_copy(
                                    out=dest[hf][:, w0c - t0:w1c - t0], in_=ps)

                def sampling_thunks_for(offv, n_rows_l, ua, ub, row_of, dest,
                                        t0, t1, evac=None):
                    """Like level_sample but returns one thunk per
                    (window, hf) for software-pipelined emission."""
                    thunks = []
                    for w0c in range(t0, t1, 512):
                        for hf in range(2):
                            def unit(w0c=w0c, hf=hf):
                                w1c = w0c + 512
                                glo = int(np.searchsorted(offv[1:], w0c,
                                                          side="right"))
                                ps = psum.tile([128, 512], F32, tag="ps")
                                r = glo
                                first = True
                                while r < n_rows_l - 1 and int(offv[r]) < w1c:
                                    s0 = max(int(offv[r]), w0c)
                                    s1 = min(int(offv[r + 1]), w1c)
                                    if s1 > s0:
                                        la, lb = s0 - w0c, s1 - w0c
                                        if ub is not None:
                                            nc.tensor.matmul(
                                                out=ps[:, la:lb],
                                                lhsT=row_of(r)[:, hf * 128:(hf + 1) * 128],
                                                rhs=ua[:, s0 - t0:s1 - t0],
                                                start=True, stop=False)
                                            nc.tensor.matmul(
                                                out=ps[:, la:lb],
                                                lhsT=row_of(r + 1)[:, hf * 128:(hf + 1) * 128],
                                                rhs=ub[:, s0 - t0:s1 - t0],
                                                start=False, stop=True)
                                        else:
                                            nc.tensor.matmul(
                                                out=ps[:, la:lb],
                                                lhsT=row_of(r)[:, hf * 128:(hf + 1) * 128],
                                                rhs=ua[:, s0 - t0:s1 - t0],
                                                start=True, stop=True)
                                        first = False
                                    r += 1
                                if first:
                                    return
                                if evac == "scalar":
                                    nc.scalar.copy(
                                        out=dest[hf][:, w0c - t0:w1c - t0],
                                        in_=ps)
                                else:
                                    nc.vector.tensor_copy(
                                        out=dest[hf][:, w0c - t0:w1c - t0],
                                        in_=ps)
                            thunks.append(unit)
                    return thunks

                with tc.tile_pool(name="pk", bufs=3) as pk_pool, \
                     tc.tile_pool(name="s0c", bufs=2) as s0c_pool, \
                     tc.tile_pool(name="schunk", bufs=2) as schunk, \
                     tc.tile_pool(name="hchunk", bufs=2) as hchunk, \
                     tc.tile_pool(name="ochunk", bufs=2) as ochunk:
                    maxc = max(e["cols"] for e in layout)
                    n_sub = TOK_CHUNK // 512

                    def issue_dma(c, base):
                        e = layout[c]
                        pk = pk_pool.tile([128, maxc], F16, tag="pk", name="pk")
                        for sa, sb in e["dma_ranges"]:
                            nc.sync.dma_start(
                                out=pk[:, sa:sb],
                                in_=pack[:, base + sa:base + sb])
                        if c == 0:
                            nc.sync.dma_start(out=w_sb, in_=wpack[:, :])
                            nc.sync.dma_start(out=b_sb, in_=bpack[:, :])
                        return pk

                    def build_sampling(c, pk):
                        """Allocate dest tiles and build sampling thunks
                        for chunk c (data already streaming into pk)."""
                        e = layout[c]
                        t0, t1 = e["t0"], e["t1"]
                        ua0 = pk[:, e["offua"]:e["offua"] + TOK_CHUNK]
                        ub0 = pk[:, e["offub"]:e["offub"] + TOK_CHUNK]
                        u1v = pk[:, e["offu1"]:e["offu1"] + TOK_CHUNK]
                        u2v = pk[0:64, e["offu2"]:e["offu2"] + TOK_CHUNK]
                        encorc = pk[64:109, e["offeo"]:e["offeo"] + TOK_CHUNK]

                        def row0(r, e=e, pk=pk):
                            i = r - e["lo0"]
                            return pk[:, e["off0"] + i * 256:e["off0"] + (i + 1) * 256]

                        def row1(r, e=e, pk=pk):
                            i = r - e["lo1"]
                            return pk[:, e["off1"] + i * 256:e["off1"] + (i + 1) * 256]

                        def row2(r, e=e, pk=pk):
                            i = r - e["lo2"]
                            return pk[0:64, e["off2"] + i * 256:e["off2"] + (i + 1) * 256]

                        s0t = [s0c_pool.tile([128, TOK_CHUNK], F16,
                                             tag=f"s0t{hf}", name=f"s0t{hf}")
                               for hf in range(2)]
                        s_sb = {}
                        for lvl in (1, 2):
                            for hf in range(2):
                                s_sb[(lvl, hf)] = schunk.tile(
                                    [128, TOK_CHUNK], F16, tag=f"s{lvl}{hf}",
                                    name=f"s{lvl}{hf}")
                        thunks = (
                            sampling_thunks_for(o0, H, ua0, ub0, row0,
                                                {0: s0t[0], 1: s0t[1]}, t0, t1)
                            + sampling_thunks_for(o1, H // 2, u1v, None, row1,
                                                  {0: s_sb[(1, 0)], 1: s_sb[(1, 1)]},
                                                  t0, t1)
                            + sampling_thunks_for(o2, H // 4, u2v, None, row2,
                                                  {0: s_sb[(2, 0)], 1: s_sb[(2, 1)]},
                                                  t0, t1, evac="scalar"))
                        l1_rhs = [encorc, s0t[0], s0t[1], s_sb[(1, 0)],
                                  s_sb[(1, 1)], s_sb[(2, 0)], s_sb[(2, 1)]]
                        return thunks, (t0, t1, l1_rhs)

                    def build_mlp(ctx):
                        """Return MLP thunk list for a sampled chunk."""
                        t0, t1, l1_rhs = ctx
                        h_cur = [hchunk.tile([128, TOK_CHUNK], F16,
                                             tag=f"h{hf}", name=f"h{hf}")
                                 for hf in range(2)]
                        thunks = []
                        for hf in range(2):
                            for s in range(n_sub):
                                def unit(hf=hf, s=s, h_cur=h_cur, l1_rhs=l1_rhs):
                                    ps = psum_mlp.tile([128, 512], F32, tag="ps")
                                    for k in range(7):
                                        nc.tensor.matmul(
                                            out=ps,
                                            lhsT=w0_parts[k][:, hf * 128:(hf + 1) * 128],
                                            rhs=l1_rhs[k][:, s * 512:(s + 1) * 512],
                                            start=(k == 0), stop=(k == 6))
                                    nc.scalar.activation(
                                        out=h_cur[hf][:, s * 512:(s + 1) * 512],
                                        in_=ps, func=GELU,
                                        bias=b0_sb[:, hf:hf + 1])
                                thunks.append(unit)
                        for layer in range(DEPTH - 1):
                            h_nxt = [hchunk.tile([128, TOK_CHUNK], F16,
                                                 tag=f"hn{layer % 2}{hf}",
                                                 name=f"hn{layer % 2}{hf}")
                                     for hf in range(2)]
                            for hf in range(2):
                                for s in range(n_sub):
                                    def unit(layer=layer, hf=hf, s=s,
                                             h_cur=h_cur, h_nxt=h_nxt):
                                        ps = psum_mlp.tile([128, 512], F32,
                                                           tag="ps")
                                        for k in range(2):
                                            nc.tensor.matmul(
                                                out=ps,
                                                lhsT=wh_parts[layer][k][:, hf * 128:(hf + 1) * 128],
                                                rhs=h_cur[k][:, s * 512:(s + 1) * 512],
                                                start=(k == 0), stop=(k == 1))
                                        nc.scalar.activation(
                                            out=h_nxt[hf][:, s * 512:(s + 1) * 512],
                                            in_=ps, func=GELU,
                                            bias=bh_sb[:, layer * 2 + hf:layer * 2 + hf + 1])
                                    thunks.append(unit)
                            h_cur = h_nxt
                        oc = ochunk.tile([3, TOK_CHUNK], F32, tag="oc")
                        for s in range(n_sub):
                            def unit(s=s, h_cur=h_cur, oc=oc, t0=t0, t1=t1):
                                ps = psum_mlp.tile([3, 512], F32, tag="ps")
                                for k in range(2):
                                    nc.tensor.matmul(
                                        out=ps, lhsT=wout_parts[k][0:128, :],
                                        rhs=h_cur[k][:, s * 512:(s + 1) * 512],
                                        start=(k == 0), stop=(k == 1))
                                nc.vector.tensor_copy(
                                    out=oc[:, s * 512:(s + 1) * 512], in_=ps)
                                if s == n_sub - 1:
                                    nc.sync.dma_start(out=out_t[:, t0:t1],
                                                      in_=oc)
                            thunks.append(unit)
                        return thunks

                    def weave(samp, mlp):
                        """Emit mlp units (prev chunk) interleaved with
                        sampling units (current chunk) so the in-order PE
                        queue has filler during activation latency."""
                        for u in mlp:
                            u()
                        for u in samp:
                            u()

                    bases = [0]
                    for e in layout:
                        bases.append(bases[-1] + e["cols"])
                    prev_mlp = []
                    pks = {0: issue_dma(0, bases[0])}
                    for c in range(n_chunks):
                        if c + 1 < n_chunks:
                            pks[c + 1] = issue_dma(c + 1, bases[c + 1])
                        samp, ctx = build_sampling(c, pks.pop(c))
                        weave(samp, prev_mlp)
                        prev_mlp = build_mlp(ctx)
                    weave([], prev_mlp)

    nc.compile()
    return nc


# ------------------------------------------------------------------ wrapper
_cache = {}


def kernel(**inputs) -> np.ndarray:
    per_core, perm, offs, totc = _host_prep(inputs)
    key = (totc,) + tuple(tuple(int(v) for v in o) for o in offs)
    if key not in _cache:
        _cache.clear()
        _cache[key] = _build_program(offs, totc)
    nc = _cache[key]
    res = run_bass_kernel_spmd(nc, per_core, core_ids=list(range(NCORES)))
    out = np.zeros((B, N, 3), np.float32)
    out_b = np.asarray(inputs["out_b"], np.float32)
    for b in range(B):
        # device returns pre-activation z = h3 @ wout; finish on host
        out[b, perm] = np.tanh(res.results[b]["out_t"].T + out_b[None, :])
    return out


if __name__ == "__main__":
    rng = np.random.default_rng(0)
    inputs = {
        "feature_grid": rng.standard_normal((B, H, W, D), dtype=np.float32),
        "context_vector": rng.standard_normal((B, D), dtype=np.float32),
        "coords": rng.uniform(-1, 1, (N, 2)).astype(np.float32),
        "oracle_pixels": rng.uniform(0, 1, (B, N, 3)).astype(np.float32),
        "mlp0_w": (rng.standard_normal((813, 256)) / np.sqrt(813)).astype(np.float32),
        "mlp0_b": np.zeros(256, np.float32),
        "mlp_hw": (rng.standard_normal((3, 256, 256)) / 16).astype(np.float32),
        "mlp_hb": np.zeros((3, 256), np.float32),
        "ctx_w": (rng.standard_normal((256, 512)) / 16).astype(np.float32),
        "ctx_b": np.zeros(512, np.float32),
        "out_w": (rng.standard_normal((256, 3)) / 16 * 0.01).astype(np.float32),
        "out_b": np.zeros(3, np.float32),
    }
    out = kernel(**inputs)
    print("kernel out:", out.shape, out.dtype, np.abs(out).max())
